# revision 34
# baseline (speedup 1.0000x reference)
"""Trainium2 Bass kernel for PointNet4DTemporalFusion (mamba+attn+ffn x5).

Sharding: pure data-parallel over batch B=8 across the 8 NeuronCores.
Each core runs the full model on one batch element. All weights are
host-transposed to [K, M] (contraction-major) bf16 so they load directly
as matmul lhsT tiles; activations are kept feature-major [feat_part, t].

Self-contained: hardcodes all shapes; host-side numpy does the shard/
transpose/cast prep and the final unshard.
"""

import numpy as np
import ml_dtypes

import concourse.bass as bass
from concourse import bacc
import concourse.mybir as mybir
import concourse.tile as tile
from concourse.bass_utils import run_bass_kernel_spmd

# ---- problem constants (hardcoded) ----
DIM = 1024
DEPTH = 5
HEADS = 4
DIM_HEAD = 1024
INNER = HEADS * DIM_HEAD            # 4096
MLP = 4 * DIM                       # 4096
B, T = 8, 150
D_IN = 2 * DIM                      # 2048
D_STATE = 16
D_CONV = 4
DT_RANK = DIM // 16                 # 64
LN_EPS = 1e-5

P = 128
NCH = DIM // P                      # 8 feature chunks of residual
NCD = D_IN // P                     # 16 channel chunks of mamba
F32 = mybir.dt.float32
BF16 = mybir.dt.bfloat16
AF = mybir.ActivationFunctionType
OP = mybir.AluOpType
AX = mybir.AxisListType
BF16NP = ml_dtypes.bfloat16

TGAP = T + 1                        # scan block with zero-gap column
NEG = -1e30


def _declare_inputs(nc, depth):
    d = {}
    def inp(name, shape, dt=F32):
        d[name] = nc.dram_tensor(name, list(shape), dt, kind="ExternalInput")
    inp("features", (P, NCH, T))
    inp("ln_w", (depth, P, 3, NCH))
    inp("ln_b", (depth, P, 3, NCH))
    inp("in_proj_wT", (depth, DIM, 2 * D_IN), BF16)
    inp("conv_w", (depth, P, NCD, D_CONV))
    inp("conv_b", (depth, P, NCD))
    inp("x_proj_wT", (depth, D_IN, 96), BF16)
    inp("dt_proj_wT", (depth, P, D_IN), BF16)      # zero-padded 64->128
    inp("dt_proj_b", (depth, P, NCD))
    inp("A_neg", (depth, P, NCD, D_STATE))
    inp("D_skip", (depth, P, NCD))
    inp("mamba_out_wT", (depth, D_IN, DIM), BF16)
    inp("qkv_wT", (depth, DIM, 3 * INNER), BF16)
    inp("attn_out_wT", (depth, INNER, DIM), BF16)
    inp("attn_out_b", (depth, P, NCH))
    inp("ffn_w1T", (depth, DIM, MLP), BF16)
    inp("ffn_b1", (depth, P, MLP // P))
    inp("ffn_w2T", (depth, MLP, DIM), BF16)
    inp("ffn_b2", (depth, P, NCH))
    inp("mask", (P, 2, T))
    inp("identity", (P, P), BF16)
    inp("ones_col", (P, 1), BF16)
    inp("ones_row", (1, P), BF16)
    inp("sel", (32, 2 * D_STATE * P), BF16)   # sel[p, n*P+m] = (p == n)
    d["out"] = nc.dram_tensor("out", [P, NCH, T], F32, kind="ExternalOutput")
    return d


def build_nc(depth=DEPTH):
    nc = bacc.Bacc(None, target_bir_lowering=False)
    io = _declare_inputs(nc, depth)

    with tile.TileContext(nc) as tc:
        with (
            tc.tile_pool(name="persist", bufs=1) as persist,
            tc.tile_pool(name="lay", bufs=1) as lay,
            tc.tile_pool(name="sc", bufs=2) as sc,
            tc.tile_pool(name="wp", bufs=3) as wp,
            tc.tile_pool(name="ps", bufs=2, space="PSUM") as ps,
        ):
            x_sb = persist.tile([P, NCH, T], F32, tag="x_sb", name="x_sb")
            nc.sync.dma_start(x_sb[:], io["features"][:])
            ident = persist.tile([P, P], BF16, tag="ident", name="ident")
            nc.sync.dma_start(ident[:], io["identity"][:])
            mask_sb = persist.tile([P, 2, T], F32, tag="mask_sb", name="mask_sb")
            nc.sync.dma_start(mask_sb[:], io["mask"][:])
            ones_col = persist.tile([P, 1], BF16, tag="ones_col",
                                    name="ones_col")
            nc.sync.dma_start(ones_col[:], io["ones_col"][:])
            eps_col = persist.tile([P, 1], F32, tag="eps_col", name="eps_col")
            nc.vector.memset(eps_col[:], LN_EPS)
            sel_sb = persist.tile([32, 2 * D_STATE * P], BF16, tag="sel_sb",
                                  name="sel_sb")
            nc.sync.dma_start(sel_sb[:], io["sel"][:])
            ones_row = persist.tile([1, P], BF16, tag="ones_row",
                                    name="ones_row")
            nc.sync.dma_start(ones_row[:], io["ones_row"][:])

            def psum_tile(j, shape=(P, T), tag=None, dtype=F32):
                tag = tag or f"ps{j % 4}"
                return ps.tile(list(shape), dtype, tag=tag, name=tag)

            # ---------- generic matmul group: out = wT.T @ rhs ----------
            def mm_group(wT_dram, K, M, rhs_fn, epilogue, wdtype=BF16,
                         m_blk=4, tag="mm"):
                assert K % P == 0
                nK = K // P
                m0 = 0
                while m0 < M:
                    mcnt = min(m_blk * P, M - m0)
                    nmb = (mcnt + P - 1) // P
                    psums = []
                    for j in range(nmb):
                        jcnt = min(P, mcnt - j * P)
                        psums.append(psum_tile(j, (jcnt, T)))
                    for k in range(nK):
                        wt = wp.tile([P, mcnt], wdtype, tag=f"{tag}_w",
                                     name=f"{tag}_w")
                        nc.sync.dma_start(
                            wt[:], wT_dram[k * P:(k + 1) * P, m0:m0 + mcnt])
                        for j in range(nmb):
                            jcnt = min(P, mcnt - j * P)
                            nc.tensor.matmul(
                                psums[j][:], wt[:, j * P:j * P + jcnt],
                                rhs_fn(k), start=(k == 0), stop=(k == nK - 1))
                    for j in range(nmb):
                        jcnt = min(P, mcnt - j * P)
                        epilogue((m0 + j * P) // P, jcnt, psums[j])
                    m0 += mcnt

            # ---------- layernorm (feature-major, partition reduce) ------
            def layer_norm(w_ap, b_ap, out_bf):
                xb = sc.tile([P, NCH, T], BF16, tag="ln_xb", name="ln_xb")
                nc.vector.tensor_copy(xb[:], x_sb[:])
                sq = sc.tile([P, NCH, T], BF16, tag="ln_sq", name="ln_sq")
                nc.scalar.square(sq[:], x_sb[:])
                psm = psum_tile(0, (1, T))
                psv = psum_tile(1, (1, T))
                for c in range(NCH):
                    nc.tensor.matmul(psm[:], ones_col[:], xb[:, c, :],
                                     start=(c == 0), stop=(c == NCH - 1))
                for c in range(NCH):
                    nc.tensor.matmul(psv[:], ones_col[:], sq[:, c, :],
                                     start=(c == 0), stop=(c == NCH - 1))
                mu = sc.tile([1, T], F32, tag="ln_mu", name="ln_mu")
                nc.vector.tensor_scalar_mul(mu[:], psm[:], 1.0 / DIM)
                mu_b = sc.tile([1, T], BF16, tag="ln_mub", name="ln_mub")
                nc.vector.tensor_copy(mu_b[:], mu[:])
                ex2 = sc.tile([1, T], F32, tag="ln_ex2", name="ln_ex2")
                nc.vector.tensor_scalar_mul(ex2[:], psv[:], 1.0 / DIM)
                musq = sc.tile([1, T], F32, tag="ln_musq", name="ln_musq")
                nc.vector.tensor_mul(musq[:], mu[:], mu[:])
                var = sc.tile([1, T], F32, tag="ln_var", name="ln_var")
                nc.vector.tensor_sub(var[:], ex2[:], musq[:])
                sd = sc.tile([1, T], F32, tag="ln_sd", name="ln_sd")
                nc.scalar.activation(sd[:], var[:], AF.Sqrt,
                                     bias=eps_col[0:1, :])
                rstd = sc.tile([1, T], BF16, tag="ln_rstd", name="ln_rstd")
                with nc.allow_low_precision(reason="bf16 rstd for PE bcast"):
                    nc.vector.reciprocal(rstd[:], sd[:])
                mu_bc = sc.tile([P, T], F32, tag="ln_mubc", name="ln_mubc")
                bcp = psum_tile(2)
                nc.tensor.matmul(bcp[:], ones_row[:], mu_b[:],
                                 start=True, stop=True)
                nc.scalar.copy(mu_bc[:], bcp[:])
                rstd_bc = sc.tile([P, T], F32, tag="ln_rbc", name="ln_rbc")
                bcp2 = psum_tile(3)
                nc.tensor.matmul(bcp2[:], ones_row[:], rstd[:],
                                 start=True, stop=True)
                nc.scalar.copy(rstd_bc[:], bcp2[:])
                for c in range(NCH):
                    xm = sc.tile([P, T], F32, tag="ln_xm", name="ln_xm")
                    nc.vector.tensor_sub(xm[:], x_sb[:, c, :], mu_bc[:])
                    nc.vector.tensor_mul(xm[:], xm[:], rstd_bc[:])
                    nc.vector.tensor_scalar(
                        out_bf[:, c, :], xm[:], w_ap[:, c:c + 1],
                        b_ap[:, c:c + 1], op0=OP.mult, op1=OP.add)

            # ================= layers =================
            for li in range(depth):
                ln_w = lay.tile([P, 3, NCH], F32, tag="ln_w", name="ln_w")
                ln_b = lay.tile([P, 3, NCH], F32, tag="ln_b", name="ln_b")
                nc.sync.dma_start(ln_w[:], io["ln_w"][li])
                nc.sync.dma_start(ln_b[:], io["ln_b"][li])

                x_ln = lay.tile([P, NCH, T], BF16, tag="x_ln", name="x_ln")

                # ---------------- Mamba ----------------
                layer_norm(ln_w[:, 0, :], ln_b[:, 0, :], x_ln)

                xc_pad = lay.tile([P, NCD, D_CONV - 1 + T], F32,
                                  tag="xc_pad", name="xc_pad")
                nc.vector.memset(xc_pad[:, :, 0:D_CONV - 1], 0.0)
                z_sb = lay.tile([P, NCD, T], F32, tag="z_sb", name="z_sb")

                def ip_epi(mc, jcnt, psum):
                    if mc < NCD:
                        nc.scalar.copy(xc_pad[:, mc, D_CONV - 1:], psum[:])
                    else:
                        nc.scalar.copy(z_sb[:, mc - NCD, :], psum[:])
                mm_group(io["in_proj_wT"][li], DIM, 2 * D_IN,
                         lambda k: x_ln[:, k, :], ip_epi, tag="ip")

                # conv1d + silu
                convw = lay.tile([P, NCD, D_CONV], F32, tag="convw",
                                 name="convw")
                nc.sync.dma_start(convw[:], io["conv_w"][li])
                convb = lay.tile([P, NCD], F32, tag="convb", name="convb")
                nc.sync.dma_start(convb[:], io["conv_b"][li])
                xs_f = lay.tile([P, NCD, T], F32, tag="xs_f", name="xs_f")
                xs_b = lay.tile([P, NCD, T], BF16, tag="xs_b", name="xs_b")
                for c in range(NCD):
                    acc = sc.tile([P, T], F32, tag="cv_acc", name="cv_acc")
                    nc.vector.tensor_scalar_mul(
                        acc[:], xc_pad[:, c, 0:T], convw[:, c, 0:1])
                    for j in range(1, D_CONV):
                        nc.vector.scalar_tensor_tensor(
                            acc[:], xc_pad[:, c, j:j + T], convw[:, c, j:j + 1],
                            acc[:], op0=OP.mult, op1=OP.add)
                    nc.scalar.activation(xs_f[:, c, :], acc[:], AF.Silu,
                                         bias=convb[:, c:c + 1])
                    nc.scalar.activation(xs_b[:, c, :], acc[:], AF.Silu,
                                         bias=convb[:, c:c + 1])

                # x_proj -> x_dbl [96, T]
                xdbl_bf = lay.tile([P, T], BF16, tag="xdbl_bf", name="xdbl_bf")
                nc.vector.memset(xdbl_bf[96:128, :], 0.0)

                def xp_epi(mc, jcnt, psum):
                    nc.scalar.copy(xdbl_bf[0:96, :], psum[:])
                mm_group(io["x_proj_wT"][li], D_IN, 96,
                         lambda k: xs_b[:, k, :], xp_epi, m_blk=1, tag="xp")
                xdbl_b = lay.tile([32, T], BF16, tag="xdbl_b", name="xdbl_b")
                nc.sync.dma_start(xdbl_b[:], xdbl_bf[64:96, :])

                # dt_proj (fp32, K padded to 128) + softplus
                dtb = lay.tile([P, NCD], F32, tag="dtb", name="dtb")
                nc.sync.dma_start(dtb[:], io["dt_proj_b"][li])
                dt_sb = lay.tile([P, NCD, T], F32, tag="dt_sb", name="dt_sb")

                def dt_epi(mc, jcnt, psum):
                    # softplus(x+b) = ln(1 + exp(x+b)); Softplus has no
                    # ACT table in this compiler build.
                    dte = sc.tile([P, T], F32, tag="dt_e", name="dt_e")
                    nc.scalar.activation(dte[:], psum[:], AF.Exp,
                                         bias=dtb[:, mc:mc + 1])
                    nc.scalar.activation(dt_sb[:, mc, :], dte[:], AF.Ln,
                                         bias=1.0)
                mm_group(io["dt_proj_wT"][li], P, D_IN,
                         lambda k: xdbl_bf[:], dt_epi, tag="dtp")

                # broadcast B and C rows across partitions via select-matmul
                B_bc = lay.tile([P, D_STATE, T], BF16, tag="B_bc", name="B_bc")
                C_bc = lay.tile([P, D_STATE, T], BF16, tag="C_bc", name="C_bc")
                for n in range(2 * D_STATE):
                    bc = psum_tile(n)
                    nc.tensor.matmul(bc[:], sel_sb[:, n * P:(n + 1) * P],
                                     xdbl_b[:], start=True, stop=True)
                    dst = (B_bc[:, n, :] if n < D_STATE
                           else C_bc[:, n - D_STATE, :])
                    nc.scalar.copy(dst, bc[:])

                A_sb = lay.tile([P, NCD, D_STATE], F32, tag="A_sb", name="A_sb")
                nc.sync.dma_start(A_sb[:], io["A_neg"][li])
                D_sb = lay.tile([P, NCD], F32, tag="D_sb", name="D_sb")
                nc.sync.dma_start(D_sb[:], io["D_skip"][li])

                ym = lay.tile([P, NCD, T], BF16, tag="ym", name="ym")

                # selective scan per channel chunk
                for c in range(NCD):
                    dtxc = sc.tile([P, T], BF16, tag="ss_dtxc", name="ss_dtxc")
                    nc.vector.tensor_mul(dtxc[:], dt_sb[:, c, :], xs_f[:, c, :])
                    dA = sc.tile([P, D_STATE, TGAP], BF16, tag="ss_dA",
                                 name="ss_dA")
                    nc.vector.tensor_tensor(
                        dA[:, :, 0:T],
                        A_sb[:, c, :, None].to_broadcast((P, D_STATE, T)),
                        dt_sb[:, c:c + 1, :].to_broadcast((P, D_STATE, T)),
                        op=OP.mult)
                    nc.vector.memset(dA[:, :, T:TGAP], 0.0)
                    nc.scalar.activation(dA[:, :, 0:T], dA[:, :, 0:T], AF.Exp)
                    dBu = sc.tile([P, D_STATE, TGAP], BF16, tag="ss_dBu",
                                  name="ss_dBu")
                    nc.vector.tensor_tensor(
                        dBu[:, :, 0:T], B_bc[:],
                        dtxc[:, None, :].to_broadcast((P, D_STATE, T)),
                        op=OP.mult)
                    nc.vector.memset(dBu[:, :, T:TGAP], 0.0)
                    hst = sc.tile([P, D_STATE, TGAP], BF16, tag="ss_h",
                                  name="ss_h")
                    nc.vector.tensor_tensor_scan(
                        hst[:].rearrange("p a b -> p (a b)"),
                        dA[:].rearrange("p a b -> p (a b)"),
                        dBu[:].rearrange("p a b -> p (a b)"),
                        0.0, op0=OP.mult, op1=OP.add)
                    prod = sc.tile([P, D_STATE, T], BF16, tag="ss_prod",
                                   name="ss_prod")
                    nc.vector.tensor_tensor(
                        prod[:], hst[:, :, 0:T], C_bc[:], op=OP.mult)
                    y_f = sc.tile([P, T], F32, tag="ss_y", name="ss_y")
                    nc.vector.tensor_reduce(
                        y_f[:], prod[:].rearrange("p n t -> p t n"),
                        axis=AX.X, op=OP.add)
                    y2 = sc.tile([P, T], F32, tag="ss_y2", name="ss_y2")
                    nc.vector.scalar_tensor_tensor(
                        y2[:], xs_f[:, c, :], D_sb[:, c:c + 1], y_f[:],
                        op0=OP.mult, op1=OP.add)
                    zs = sc.tile([P, T], F32, tag="ss_zs", name="ss_zs")
                    nc.scalar.activation(zs[:], z_sb[:, c, :], AF.Silu)
                    nc.vector.tensor_mul(ym[:, c, :], y2[:], zs[:])

                # mamba out proj + residual add
                def mo_epi(mc, jcnt, psum):
                    nc.vector.tensor_add(x_sb[:, mc, :], x_sb[:, mc, :],
                                         psum[:])
                mm_group(io["mamba_out_wT"][li], D_IN, DIM,
                         lambda k: ym[:, k, :], mo_epi, tag="mo")

                # ---------------- Attention ----------------
                layer_norm(ln_w[:, 1, :], ln_b[:, 1, :], x_ln)
                qkvT = io["qkv_wT"][li]
                av_sb = lay.tile([P, INNER // P, T], BF16, tag="av_sb",
                                 name="av_sb")
                for h in range(HEADS):
                    qoff = h * DIM_HEAD
                    koff = INNER + h * DIM_HEAD
                    voff = 2 * INNER + h * DIM_HEAD
                    q_sb = lay.tile([P, NCH, T], BF16, tag="q_sb", name="q_sb")
                    k_sb = lay.tile([P, NCH, T], BF16, tag="k_sb", name="k_sb")

                    def q_epi(mc, jcnt, psum):
                        nc.vector.tensor_scalar_mul(
                            q_sb[:, mc, :], psum[:], DIM_HEAD ** -0.5)
                    mm_group(qkvT[:, qoff:qoff + DIM_HEAD], DIM, DIM_HEAD,
                             lambda k: x_ln[:, k, :], q_epi, tag="qm")

                    def k_epi(mc, jcnt, psum):
                        nc.scalar.copy(k_sb[:, mc, :], psum[:])
                    mm_group(qkvT[:, koff:koff + DIM_HEAD], DIM, DIM_HEAD,
                             lambda k: x_ln[:, k, :], k_epi, tag="km")

                    # v token-major: [tok, dv]
                    v_sb = lay.tile([P, 2, DIM_HEAD], BF16, tag="v_sb",
                                    name="v_sb")
                    nc.vector.memset(v_sb[:, 1, :], 0.0)
                    for dv in range(2):
                        vps = [psum_tile(2 * dv + tj, (P if tj == 0 else T - P,
                                                       512))
                               for tj in range(2)]
                        for k in range(NCH):
                            wv = wp.tile([P, 512], BF16, tag="wv", name="wv")
                            nc.sync.dma_start(
                                wv[:], qkvT[k * P:(k + 1) * P,
                                            voff + dv * 512:voff + (dv + 1) * 512])
                            for tj in range(2):
                                lh = (x_ln[:, k, 0:P] if tj == 0
                                      else x_ln[:, k, P:T])
                                nc.tensor.matmul(vps[tj][:], lh, wv[:],
                                                 start=(k == 0),
                                                 stop=(k == NCH - 1))
                        for tj in range(2):
                            tcnt = P if tj == 0 else T - P
                            nc.scalar.copy(
                                v_sb[0:tcnt, tj, dv * 512:(dv + 1) * 512],
                                vps[tj][:])

                    # dots + causal softmax -> P_sb [i_part, ic, j]
                    P_sb = lay.tile([P, 2, T], BF16, tag="P_sb", name="P_sb")
                    for ic in range(2):
                        icnt = P if ic == 0 else T - P
                        dps = psum_tile(ic, (icnt, T))
                        for k in range(NCH):
                            nc.tensor.matmul(
                                dps[:], q_sb[:, k, ic * P:ic * P + icnt],
                                k_sb[:, k, :], start=(k == 0),
                                stop=(k == NCH - 1))
                        df = sc.tile([P, T], F32, tag="at_df", name="at_df")
                        nc.vector.tensor_add(df[0:icnt, :], dps[:],
                                             mask_sb[0:icnt, ic, :])
                        mx = sc.tile([P, 1], F32, tag="at_mx", name="at_mx")
                        nc.vector.tensor_reduce(mx[0:icnt, :], df[0:icnt, :],
                                                axis=AX.X, op=OP.max)
                        nmx = sc.tile([P, 1], F32, tag="at_nmx", name="at_nmx")
                        nc.vector.tensor_scalar_mul(nmx[0:icnt, :],
                                                    mx[0:icnt, :], -1.0)
                        pex = sc.tile([P, T], F32, tag="at_pex", name="at_pex")
                        sme = sc.tile([P, 1], F32, tag="at_sme", name="at_sme")
                        nc.scalar.activation(pex[0:icnt, :], df[0:icnt, :],
                                             AF.Exp, bias=nmx[0:icnt, 0:1],
                                             accum_out=sme[0:icnt, 0:1])
                        rs = sc.tile([P, 1], F32, tag="at_rs", name="at_rs")
                        nc.vector.reciprocal(rs[0:icnt, :], sme[0:icnt, :])
                        nc.vector.tensor_scalar_mul(
                            P_sb[0:icnt, ic, :], pex[0:icnt, :],
                            rs[0:icnt, 0:1])

                    # P^T via PE transposes
                    PT_sb = lay.tile([P, 2, T], BF16, tag="PT_sb", name="PT_sb")
                    nc.vector.memset(PT_sb[:, 1, :], 0.0)
                    for ic in range(2):
                        icnt = P if ic == 0 else T - P
                        for jc in range(2):
                            jcnt = P if jc == 0 else T - P
                            tp = psum_tile(2 * ic + jc, (jcnt, icnt),
                                           dtype=BF16)
                            nc.tensor.transpose(
                                tp[:], P_sb[0:icnt, ic, jc * P:jc * P + jcnt],
                                ident[0:icnt, 0:icnt])
                            nc.scalar.copy(
                                PT_sb[0:jcnt, jc, ic * P:ic * P + icnt], tp[:])

                    # av^T = sum_j v[j, dv] * P^T[j, i]  (feature-major out)
                    for mc in range(NCH):
                        aps = psum_tile(mc, (P, T))
                        for jc in range(2):
                            nc.tensor.matmul(
                                aps[:], v_sb[:, jc, mc * P:(mc + 1) * P],
                                PT_sb[:, jc, :], start=(jc == 0),
                                stop=(jc == 1))
                        nc.scalar.copy(av_sb[:, h * NCH + mc, :], aps[:])

                # attn out proj + gelu + residual
                aob = lay.tile([P, NCH], F32, tag="aob", name="aob")
                nc.sync.dma_start(aob[:], io["attn_out_b"][li])

                def ao_epi(mc, jcnt, psum):
                    tf = sc.tile([P, T], F32, tag="ao_tf", name="ao_tf")
                    nc.scalar.activation(tf[:], psum[:], AF.Gelu,
                                         bias=aob[:, mc:mc + 1])
                    nc.vector.tensor_add(x_sb[:, mc, :], x_sb[:, mc, :], tf[:])
                mm_group(io["attn_out_wT"][li], INNER, DIM,
                         lambda k: av_sb[:, k, :], ao_epi, tag="ao")

                # ---------------- FFN ----------------
                layer_norm(ln_w[:, 2, :], ln_b[:, 2, :], x_ln)
                fb1 = lay.tile([P, MLP // P], F32, tag="fb1", name="fb1")
                nc.sync.dma_start(fb1[:], io["ffn_b1"][li])
                fb2 = lay.tile([P, NCH], F32, tag="fb2", name="fb2")
                nc.sync.dma_start(fb2[:], io["ffn_b2"][li])
                h_sb = lay.tile([P, MLP // P, T], BF16, tag="h_sb", name="h_sb")

                def f1_epi(mc, jcnt, psum):
                    nc.scalar.activation(h_sb[:, mc, :], psum[:], AF.Gelu,
                                         bias=fb1[:, mc:mc + 1])
                mm_group(io["ffn_w1T"][li], DIM, MLP,
                         lambda k: x_ln[:, k, :], f1_epi, tag="f1")

                def f2_epi(mc, jcnt, psum):
                    nc.vector.scalar_tensor_tensor(
                        x_sb[:, mc, :], psum[:], fb2[:, mc:mc + 1],
                        x_sb[:, mc, :], op0=OP.add, op1=OP.add)
                mm_group(io["ffn_w2T"][li], MLP, DIM,
                         lambda k: h_sb[:, k, :], f2_epi, tag="f2")

            nc.sync.dma_start(io["out"][:], x_sb[:])

    nc.compile()
    return nc


# ================= host-side prep =================

def _feat_major(a):
    """(T, F) fp32 -> [P, F//P, T]"""
    Tn, F = a.shape
    return np.ascontiguousarray(
        a.T.reshape(F // P, P, Tn).transpose(1, 0, 2)).astype(np.float32)


def _chunk_cols(v):
    """(F,) -> [P, F//P] (chunk c in column c)"""
    return np.ascontiguousarray(v.reshape(-1, P).T).astype(np.float32)


def host_prep(inputs, depth=DEPTH):
    f32 = np.float32
    g = {k: np.asarray(v, f32) for k, v in inputs.items()}
    w = {}
    w["ln_w"] = np.stack([
        np.stack([_chunk_cols(g["mnorm_w"][i]), _chunk_cols(g["ln1_w"][i]),
                  _chunk_cols(g["ln2_w"][i])], axis=1) for i in range(depth)])
    w["ln_b"] = np.stack([
        np.stack([_chunk_cols(g["mnorm_b"][i]), _chunk_cols(g["ln1_b"][i]),
                  _chunk_cols(g["ln2_b"][i])], axis=1) for i in range(depth)])
    bf = lambda x: np.ascontiguousarray(x).astype(BF16NP)
    w["in_proj_wT"] = np.stack(
        [bf(g["in_proj_w"][i].T) for i in range(depth)])
    w["conv_w"] = np.stack([
        np.ascontiguousarray(
            g["conv_w"][i].reshape(NCD, P, D_CONV).transpose(1, 0, 2))
        for i in range(depth)])
    w["conv_b"] = np.stack([_chunk_cols(g["conv_b"][i]) for i in range(depth)])
    w["x_proj_wT"] = np.stack(
        [bf(g["x_proj_w"][i].T) for i in range(depth)])
    dtT = np.zeros((depth, P, D_IN), f32)
    for i in range(depth):
        dtT[i, :DT_RANK] = g["dt_proj_w"][i].T
    w["dt_proj_wT"] = dtT.astype(BF16NP)
    w["dt_proj_b"] = np.stack(
        [_chunk_cols(g["dt_proj_b"][i]) for i in range(depth)])
    A = -np.exp(g["A_log"])                       # (depth, D_IN, D_STATE)
    w["A_neg"] = np.stack([
        np.ascontiguousarray(
            A[i].reshape(NCD, P, D_STATE).transpose(1, 0, 2))
        for i in range(depth)]).astype(f32)
    w["D_skip"] = np.stack([_chunk_cols(g["D_skip"][i]) for i in range(depth)])
    w["mamba_out_wT"] = np.stack(
        [bf(g["mamba_out_w"][i].T) for i in range(depth)])
    w["qkv_wT"] = np.stack([bf(g["qkv_w"][i].T) for i in range(depth)])
    w["attn_out_wT"] = np.stack(
        [bf(g["attn_out_w"][i].T) for i in range(depth)])
    w["attn_out_b"] = np.stack(
        [_chunk_cols(g["attn_out_b"][i]) for i in range(depth)])
    w["ffn_w1T"] = np.stack([bf(g["ffn_w1"][i].T) for i in range(depth)])
    w["ffn_b1"] = np.stack([_chunk_cols(g["ffn_b1"][i]) for i in range(depth)])
    w["ffn_w2T"] = np.stack([bf(g["ffn_w2"][i].T) for i in range(depth)])
    w["ffn_b2"] = np.stack([_chunk_cols(g["ffn_b2"][i]) for i in range(depth)])

    mask = np.zeros((P, 2, T), f32)
    for ic in range(2):
        for p in range(P):
            i = ic * P + p
            if i < T:
                mask[p, ic, i + 1:] = NEG
    w["mask"] = mask
    w["identity"] = np.eye(P, dtype=BF16NP)
    w["ones_col"] = np.ones((P, 1), BF16NP)
    w["ones_row"] = np.ones((1, P), BF16NP)
    sel = np.zeros((32, 2 * D_STATE * P), np.float32)
    for n in range(2 * D_STATE):
        sel[n, n * P:(n + 1) * P] = 1.0
    w["sel"] = sel.astype(BF16NP)

    feats = g["features"]
    in_maps = []
    for b in range(B):
        m = dict(w)
        m["features"] = _feat_major(feats[b])
        in_maps.append(m)
    return in_maps


_NC_CACHE = {}


def _get_nc(depth=DEPTH):
    if depth not in _NC_CACHE:
        _NC_CACHE[depth] = build_nc(depth)
    return _NC_CACHE[depth]


def kernel(**inputs):
    nc = _get_nc()
    in_maps = host_prep(inputs)
    res = run_bass_kernel_spmd(nc, in_maps, core_ids=list(range(B)))
    outs = []
    for r in res.results:
        o = np.asarray(r["out"], np.float32)       # [P, NCH, T]
        outs.append(o.transpose(1, 0, 2).reshape(DIM, T).T)
    return np.ascontiguousarray(np.stack(outs)).astype(np.float32)
